# revision 1
# baseline (speedup 1.0000x reference)
"""Trainium kernel for nn_LCP (ConvPoint-style LCP layer), 8 NeuronCores.

Sharding: 8 cores = 4 batches x 2 halves of the N_support axis.
Each core receives the FULL points/support for its batch (25MB/4 -- cheap)
so InstanceNorm statistics over (N, K) are computed locally without any
cross-core collective; the heavy tensors (`input`, 537MB) and the two large
einsums are sharded over the N-half.  Output halves are concatenated on host.
"""

import numpy as np
import jax
import jax.numpy as jnp
from functools import partial

EPS_IN = 1e-5
B, C_IN, C_OUT, N, K, KSZ, DIM = 4, 64, 128, 16384, 32, 16, 3
NCORES = 8
NH = N // 2  # half of N per core


def _inorm(x, g, b):
    # x: (C, N, K) single instance
    m = x.mean(axis=(1, 2), keepdims=True)
    v = ((x - m) ** 2).mean(axis=(1, 2), keepdims=True)
    return (x - m) / jnp.sqrt(v + EPS_IN) * g[:, None, None] + b[:, None, None]


def _core_fn(input_h, points, support_points, fc1_w, fc2_w, fc3_w,
             bn1_g, bn1_b, bn2_g, bn2_b, alpha, beta, cv_w, cv_b, n0):
    # points/support: full N for this batch; input_h: (C_in, NH, K) half.
    pts = points - support_points[:, :, None]                    # (3,N,K)
    distances = jnp.sqrt((pts ** 2).sum(0))                      # (N,K)
    dw = jax.nn.sigmoid(-alpha * distances + beta)               # (N,K)
    dws = dw.sum(1, keepdims=True)
    dws = dws + (dws == 0).astype(dw.dtype) + 1e-6
    dw = dw / dws * K                                            # (N,K)

    mat = jax.nn.relu(_inorm(jnp.einsum('dnk,od->onk', pts, fc1_w),
                             bn1_g, bn1_b))                      # (16,N,K)
    mp1 = (mat * dw[None]).max(axis=2, keepdims=True)
    mat = jnp.concatenate([mat, jnp.broadcast_to(mp1, mat.shape)], axis=0)
    mat = jax.nn.relu(_inorm(jnp.einsum('cnk,oc->onk', mat, fc2_w),
                             bn2_g, bn2_b))
    mp2 = (mat * dw[None]).max(axis=2, keepdims=True)
    mat = jnp.concatenate([mat, jnp.broadcast_to(mp2, mat.shape)], axis=0)
    mat = jax.nn.relu(jnp.einsum('cnk,oc->onk', mat, fc3_w)) * dw[None]

    # shard the heavy contraction over this core's N-half
    mat_h = jax.lax.dynamic_slice_in_dim(mat, n0, NH, axis=1)    # (16,NH,K)
    feat = jnp.einsum('ink,enk->ine', input_h, mat_h)            # (C_in,NH,16)
    features = jnp.einsum('ine,oie->on', feat, cv_w) + cv_b[:, None]
    return features                                              # (C_out,NH)


_PMAPPED = None


def _get_pmapped():
    global _PMAPPED
    if _PMAPPED is None:
        devs = jax.devices()[:NCORES]
        _PMAPPED = jax.pmap(_core_fn, devices=devs)
    return _PMAPPED


def kernel(input, points, support_points, fc1_w, fc2_w, fc3_w,
           bn1_g, bn1_b, bn2_g, bn2_b, alpha, beta, cv_w, cv_b):
    input = np.asarray(input, np.float32)
    points = np.asarray(points, np.float32)
    support_points = np.asarray(support_points, np.float32)

    # per-core shards: core i -> batch i//2, half i%2
    inp_sh = np.stack([input[i // 2, :, (i % 2) * NH:(i % 2) * NH + NH, :]
                       for i in range(NCORES)])
    pts_sh = np.stack([points[i // 2] for i in range(NCORES)])
    sup_sh = np.stack([support_points[i // 2] for i in range(NCORES)])
    n0_sh = np.array([(i % 2) * NH for i in range(NCORES)], np.int32)

    def rep(x):
        x = np.asarray(x, np.float32)
        return np.broadcast_to(x, (NCORES,) + x.shape).copy()

    out = _get_pmapped()(
        inp_sh, pts_sh, sup_sh, rep(fc1_w), rep(fc2_w), rep(fc3_w),
        rep(bn1_g), rep(bn1_b), rep(bn2_g), rep(bn2_b),
        rep(alpha), rep(beta), rep(cv_w), rep(cv_b), n0_sh)
    out = np.asarray(out)                                        # (8,C_out,NH)

    features = np.empty((B, C_OUT, N), np.float32)
    for b in range(B):
        features[b, :, :NH] = out[2 * b]
        features[b, :, NH:] = out[2 * b + 1]
    return features, np.asarray(support_points, np.float32)
